# revision 56
# baseline (speedup 1.0000x reference)
"""Link-predictor GNN kernel for 8 TRN2 NeuronCores.

Strategy (per sharding hint): shard edges across 8 cores (data parallel),
replicate the bf16-cast node-embedding table + MLP weights on every core.

The gather uses the SWDGE dma_gather ucode (transpose=True), which lands
X^T = emd[idx].T directly in SBUF as [128 d, n_edges] — no PE transposes
and no PSUM->SBUF copies.  dma_gather indices are int16, so nodes are
bucketed into 4 ranges of 25000 rows and edges are classified into 16
(src_bucket, dst_bucket) classes GLOBALLY; each class's edges are dealt
round-robin across the 8 cores, so per-(core, class) counts are
n_k/8 +- 1, and each class gets an adaptive capacity ceil(n_k/8) rounded
up to 128 (the program is built at runtime for those capacities and
cached).  Pad slots use index 0; padded outputs are dropped on the host,
which un-permutes edges back to input order.

Each class-side is gathered in two half-slices into the same SBUF tile
(view-granular deps let tiles 0-4 start after the first half), and the
first/last classes are split finer to shorten the pipeline head/tail;
the last class is also processed as mini-groups with separate output
stores so the machine tail after the final gather is only a tile or two.

Per edge tile: 4 matmuls (K-blocks src/dst x h-blocks 0/1) into two PSUM
tiles, relu on ACT (h0) + DVE tensor_scalar (h1), then — software-
pipelined one tile behind so the PE never head-of-line blocks on the
relu — 2 matmuls for logits and sigmoid on ACT into a per-group output
row, one output DMA per group.

Timeline-model engine budget per core: DMA engines ~224us (gathers
~216 at the 256B-descriptor bandwidth floor + idx/weight/output
traffic), PE ~195, ACT ~187, DVE ~99, Pool desc-gen ~87; the DMA stream
is saturated end to end.  Total 237.0us (baseline: 1236.6us).
"""

import sys

sys.path.insert(0, "/opt/trn_rl_repo")

import numpy as np
import ml_dtypes

from concourse import bacc, mybir, tile
from concourse.bass_utils import run_bass_kernel_spmd

BF16 = ml_dtypes.bfloat16

N_NODES = 100000
D = 128
H = 256
E_TOTAL = 600000
NCORES = 8

NB = 25000                   # node-bucket width (int16-safe)
NBUCK = 4
NCLS = NBUCK * NBUCK         # 16 (src_bucket, dst_bucket) classes
LAST_RESULTS = None
_NC = {}


def _tiles(ck):
    widths = [512] * (ck // 512) + ([ck % 512] if ck % 512 else [])
    starts = [sum(widths[:i]) for i in range(len(widths))]
    return list(zip(starts, widths))


def _build_program(cks):
    """cks: per-class slot capacities (each a multiple of 128), adapted to
    the actual class sizes of the input at runtime."""
    global _NC
    if cks in _NC:
        return _NC[cks]
    offs = [sum(cks[:i]) for i in range(NCLS)]
    E_PAD = sum(cks)
    dt = mybir.dt
    nc = bacc.Bacc(
        "TRN2",
        target_bir_lowering=False,
        debug=False,
        enable_asserts=False,
        num_devices=NCORES,
    )
    emd = nc.dram_tensor("emd", [N_NODES, D], dt.bfloat16, kind="ExternalInput")
    sidx_d = nc.dram_tensor("sidx", [128, E_PAD // 16], dt.int16, kind="ExternalInput")
    didx_d = nc.dram_tensor("didx", [128, E_PAD // 16], dt.int16, kind="ExternalInput")
    w1_d = nc.dram_tensor("w1", [128, 512], dt.bfloat16, kind="ExternalInput")
    w2_d = nc.dram_tensor("w2", [128, 2], dt.bfloat16, kind="ExternalInput")
    b1_d = nc.dram_tensor("b1", [128, 2], dt.float32, kind="ExternalInput")
    b2_d = nc.dram_tensor("b2", [1, 1], dt.float32, kind="ExternalInput")
    out_d = nc.dram_tensor("out", [1, E_PAD], dt.float32, kind="ExternalOutput")

    AF = mybir.ActivationFunctionType
    ALU = mybir.AluOpType

    with tile.TileContext(nc) as tc:
        with (
            tc.tile_pool(name="const", bufs=1) as cpool,
            tc.tile_pool(name="x", bufs=3) as xpool,
            tc.tile_pool(name="h", bufs=3) as hpool,
            tc.tile_pool(name="o", bufs=3) as opool,
            tc.tile_pool(name="ph", bufs=2, space="PSUM") as php,
            tc.tile_pool(name="pl", bufs=2, space="PSUM") as plp,
        ):
            sidx = cpool.tile([128, E_PAD // 16], dt.int16)
            nc.sync.dma_start(sidx[:, :], sidx_d[:, :])
            didx = cpool.tile([128, E_PAD // 16], dt.int16)
            nc.sync.dma_start(didx[:, :], didx_d[:, :])
            w1_sb = cpool.tile([128, 512], dt.bfloat16)
            nc.sync.dma_start(w1_sb[:, :], w1_d[:, :])
            w2_sb = cpool.tile([128, 2], dt.bfloat16)
            nc.sync.dma_start(w2_sb[:, :], w2_d[:, :])
            b1_sb = cpool.tile([128, 2], dt.float32)
            nc.sync.dma_start(b1_sb[:, :], b1_d[:, :])
            b2_sb = cpool.tile([1, 1], dt.float32)
            nc.sync.dma_start(b2_sb[:, :], b2_d[:, :])

            # one-tile-deep software pipeline for the logits stage:
            # (h0_sb, h1_sb, o_sb, off, width, store or None) where store =
            # (class, group col range) for the group's last tile
            pending = None

            def flush(p):
                h0_sb, h1_sb, o_sb, off, w, store = p
                l_ps = plp.tile([1, w], dt.float32, tag="l")
                nc.tensor.matmul(
                    l_ps[:, :], lhsT=w2_sb[:, 0:1], rhs=h0_sb[:, :],
                    start=True, stop=False,
                )
                nc.tensor.matmul(
                    l_ps[:, :], lhsT=w2_sb[:, 1:2], rhs=h1_sb[:, :],
                    start=False, stop=True,
                )
                nc.scalar.activation(
                    o_sb[0:1, off : off + w], l_ps[:, :], AF.Sigmoid,
                    bias=b2_sb[:, 0:1],
                )
                if store is not None:
                    sa, sb_ = store
                    nc.sync.dma_start(
                        out_d[0:1, sa:sb_], o_sb[:, :]
                    )

            def gather(dst_ap, bucket, idx_tile, col0, n):
                nc.gpsimd.dma_gather(
                    out_ap=dst_ap,
                    in_ap=emd[bucket * NB : (bucket + 1) * NB, :],
                    idxs_ap=idx_tile[:, col0 : col0 + n // 16],
                    num_idxs=n,
                    num_idxs_reg=n,
                    elem_size=D,
                    transpose=True,
                    single_packet=False,
                )

            for k in range(NCLS):
                sb, db = divmod(k, NBUCK)
                ck = cks[k]
                co = offs[k]
                tiles_k = _tiles(ck)
                xs = xpool.tile([128, 1, ck], dt.bfloat16, tag="xs")
                xd = xpool.tile([128, 1, ck], dt.bfloat16, tag="xd")
                half = (ck // 256) * 128
                if k == 0:
                    # compute can start after a small first piece
                    splits = [0, 1280, ck]
                    groups = [(0, ck)]
                elif k == NCLS - 1:
                    # mini-groups with separate stores: the machine's tail
                    # after the final gather lands is only a tile or two
                    nt = len(tiles_k)
                    cuts = sorted({tiles_k[min(i, nt - 1)][0]
                                   for i in (5, 7, 8, 9)} | {0, ck})
                    splits = cuts
                    groups = list(zip(cuts, cuts[1:]))
                else:
                    # half-gathers per side: the first tiles start after xd
                    # of the first half instead of waiting for the class
                    splits = [0, half, ck]
                    groups = [(0, ck)]
                for a, b in zip(splits, splits[1:]):
                    gather(xs[:, :, a:b], sb, sidx, co // 16 + a // 16, b - a)
                    gather(xd[:, :, a:b], db, didx, co // 16 + a // 16, b - a)
                for ga, gb in groups:
                    o_sb = opool.tile([1, gb - ga], dt.float32, tag="o")
                    gtiles = [(c0, w) for c0, w in tiles_k
                              if ga <= c0 and c0 + w <= gb]
                    for c0, w in gtiles:
                        rs = xs[:, 0, c0 : c0 + w]
                        rd = xd[:, 0, c0 : c0 + w]
                        h0_ps = php.tile([128, w], dt.float32, tag="h0")
                        h1_ps = php.tile([128, w], dt.float32, tag="h1")
                        nc.tensor.matmul(
                            h0_ps[:, :], lhsT=w1_sb[:, 0:128], rhs=rs,
                            start=True, stop=False,
                        )
                        nc.tensor.matmul(
                            h0_ps[:, :], lhsT=w1_sb[:, 256:384], rhs=rd,
                            start=False, stop=True,
                        )
                        nc.tensor.matmul(
                            h1_ps[:, :], lhsT=w1_sb[:, 128:256], rhs=rs,
                            start=True, stop=False,
                        )
                        nc.tensor.matmul(
                            h1_ps[:, :], lhsT=w1_sb[:, 384:512], rhs=rd,
                            start=False, stop=True,
                        )
                        if pending is not None:
                            flush(pending)
                        h0_sb = hpool.tile([128, w], dt.bfloat16, tag="h0s")
                        h1_sb = hpool.tile([128, w], dt.bfloat16, tag="h1s")
                        nc.scalar.activation(
                            h0_sb[:, :], h0_ps[:, :], AF.Relu,
                            bias=b1_sb[:, 0:1],
                        )
                        nc.vector.tensor_scalar(
                            h1_sb[:, :], h1_ps[:, :],
                            b1_sb[:, 1:2], 0.0, ALU.add, ALU.max,
                        )
                        is_last = (c0, w) == gtiles[-1]
                        pending = (
                            h0_sb, h1_sb, o_sb, c0 - ga, w,
                            (co + ga, co + gb) if is_last else None,
                        )
            flush(pending)

    nc.compile()
    _NC[cks] = nc
    return nc


def _wrap16(flat, cks):
    """[sum(cks)] int16 -> [128, sum(cks)//16]: class k's block of ck slots
    wrapped so gather slot j reads idxs[j % 16, j // 16] within the block
    (first 16 partitions, replicated to all 8 partition groups)."""
    blocks, off = [], 0
    for ck in cks:
        blk = flat[off : off + ck]
        blocks.append(blk.reshape(ck // 16, 16).T)    # [16, ck//16]
        off += ck
    b = np.concatenate(blocks, axis=1)
    return np.ascontiguousarray(np.tile(b, (8, 1)))


def _prepare_inputs(emd_all, edge_index, W1, b1, W2, b2):
    emd_bf = np.ascontiguousarray(np.asarray(emd_all, dtype=np.float32)).astype(BF16)
    ei = np.asarray(edge_index).astype(np.int64)
    W1 = np.asarray(W1, dtype=np.float32)
    W2 = np.asarray(W2, dtype=np.float32)
    b1 = np.asarray(b1, dtype=np.float32).reshape(-1)
    b2 = np.asarray(b2, dtype=np.float32).reshape(-1)

    # lhsT blocks: cols 0:256 = W1[:128,:] (src side), 256:512 = W1[128:,:]
    w1_arr = np.concatenate([W1[:D, :], W1[D:, :]], axis=1).astype(BF16)
    w2_arr = np.stack([W2[:128, 0], W2[128:, 0]], axis=1).astype(BF16)
    b1_arr = np.ascontiguousarray(np.stack([b1[:128], b1[128:]], axis=1))
    b2_arr = b2.reshape(1, 1)

    s, d = ei[:, 0], ei[:, 1]
    kcls = (s // NB) * NBUCK + (d // NB)
    counts = np.bincount(kcls, minlength=NCLS)
    order_g = np.argsort(kcls, kind="stable")     # edges grouped by class
    ks = kcls[order_g]
    grp_start = np.zeros(NCLS, np.int64)
    grp_start[1:] = np.cumsum(counts)[:-1]
    pos = np.arange(E_TOTAL) - grp_start[ks]      # position within class
    core = pos % NCORES                           # deal round-robin to cores
    # adaptive per-class capacity: ceil(n_k / 8) rounded up to 128
    cks = tuple(int(-(-(-(-int(n) // NCORES)) // 128) * 128) for n in counts)
    offs = np.array([sum(cks[:i]) for i in range(NCLS)], np.int64)
    E_PAD = sum(cks)
    slot = offs[ks] + pos // NCORES               # slot on that core
    assert all(-(-int(n) // NCORES) <= ck for n, ck in zip(counts, cks))

    in_maps, unshard = [], []
    for c in range(NCORES):
        m = core == c
        eids = order_g[m]                         # global edge ids on core c
        slots = slot[m]
        sflat = np.zeros(E_PAD, np.int16)
        dflat = np.zeros(E_PAD, np.int16)
        sflat[slots] = (s[eids] % NB).astype(np.int16)
        dflat[slots] = (d[eids] % NB).astype(np.int16)
        in_maps.append(
            {
                "emd": emd_bf,
                "sidx": _wrap16(sflat, cks),
                "didx": _wrap16(dflat, cks),
                "w1": w1_arr,
                "w2": w2_arr,
                "b1": b1_arr,
                "b2": b2_arr,
            }
        )
        unshard.append((eids, slots))
    return in_maps, unshard, cks


def kernel(emd_all, edge_index, W1, b1, W2, b2):
    global LAST_RESULTS
    in_maps, unshard, cks = _prepare_inputs(emd_all, edge_index, W1, b1, W2, b2)
    nc = _build_program(cks)
    res = run_bass_kernel_spmd(nc, in_maps, core_ids=list(range(NCORES)))
    LAST_RESULTS = res
    out = np.empty((E_TOTAL,), dtype=np.float32)
    for c in range(NCORES):
        flat = np.asarray(res.results[c]["out"], dtype=np.float32).reshape(-1)
        eids, slots = unshard[c]
        out[eids] = flat[slots]
    return out.reshape(E_TOTAL, 1)


if __name__ == "__main__":
    rng = np.random.default_rng(0)
    emd = rng.standard_normal((N_NODES, D), dtype=np.float32)
    ei = rng.integers(0, N_NODES, size=(E_TOTAL, 2)).astype(np.int32)
    W1 = rng.standard_normal((2 * D, H), dtype=np.float32) / np.sqrt(2 * D)
    W2 = rng.standard_normal((H, 1), dtype=np.float32) / np.sqrt(H)
    out = kernel(emd, ei, W1, np.zeros(H, np.float32), W2, np.zeros(1, np.float32))
    print(out.shape, out[:4, 0])
